# revision 32
# baseline (speedup 1.0000x reference)
"""Trainium2 Bass kernel for the real-space Ewald sum (nn_Ewald) — v2.

Math (per molecule, 2048 atoms, 8 charge channels):
    pot = sum_{i,j} qq_ij * erf(|rij|/sqrt(2)) / (|rij|+1e-6) / (4*pi)
        + sum_i qq_ii / (2*pi)^1.5   (self term), scaled by NORM_FACTOR.

Per pair tile the device computes (as in v1):
    s' = delta*s  (K=13 bf16 hi/lo augmented matmul, delta=2^-3 exact)
    y  = kappa/sqrt(s)        (ACT Abs_reciprocal_sqrt -> bf16)
    w' = min(y, max(P(s'),1)) (one custom DVE op, monic cubic P -> bf16)
    u[c,j] += q[i,c] w'_ij    (PE contraction, f32 PSUM)
Host: pot = (sum(u*q^T) - sum_i qq_ii w'_ii)/kappa/(4*pi) + self term.

v2 geometry — circulant tournament cover at 128-atom granularity:
  16 blocks per molecule; slot centered at block c covers the cyclic
  block window [c, c+8] (9 blocks) for c in 0..7, [c, c+7] (8 blocks)
  for c in 8..15.  Every unordered block pair is covered exactly once
  (difference 1..7 -> lower center; difference 8 -> center < 8; diag ->
  own center with weight 1 via q1, off-center blocks weight 2 via q2).
  Total 68 blocks/core = 8704 pair-columns (vs 10240 in v1, -15%).

SPMD: every core runs slots of widths {9,9,9,9,8,8,8,8} at FIXED window
positions [128j, 128j+1152) (j=0..3) and [128(8+j), 128(8+j)+1024) over
a 19-block extended column space.  Per-core data: augm_ext columns are
the molecule's blocks in cyclic order rotated by 0 (even core: centers
0-3,8-11) or 4 (odd core: centers 4-7,12-15); augs/q12 hold the 8
center blocks.  8 cores = 4 molecules x 2 center sets.

u accumulates in 2 PSUM banks (bands of 512 columns packed at partition
offsets 0/32/64/96 via matmul tile_position); banks are memset to zero
so all matmuls accumulate (start=False).  Host folds the 19-position
extended u back onto the 16 blocks.

PE runs at 1.2 GHz on this platform (HAM clock gate never releases), so
the kernel is PE-stream-bound: A 8704 + B 8704 columns ~= 14.5 us/core.
"""

import numpy as np

B = 4
NB = 2048
NQ = 8
NCORES = 8
NSLOT = 8
NPOS = 19            # extended column space, in 128-blocks
NEXT = NPOS * 128    # 2432 extended columns
NORM_FACTOR = 90.0474
KA = 13              # augmented contraction depth (bf16 hi/lo split)

# fitted scheme constants (see v1): s_c=9, delta=2^-3
DELTA = 0.125
KAPPA = 3.0708577931200534
PC0 = 2.7432632222505378
PC1 = -3.112066562880879
PC2 = 2.4529603188601343
ACT_SCALE = 1.0 / (DELTA * KAPPA * KAPPA)

# slot windows in the extended space: (start, width) in columns
SLOT_WIN = [(128 * j, 1152) for j in range(4)] + [
    (128 * (8 + j), 1024) for j in range(4)
]
# centers per core half h (h=0: rotation 0, h=1: rotation 4)
CENTERS = {0: [0, 1, 2, 3, 8, 9, 10, 11], 1: [4, 5, 6, 7, 12, 13, 14, 15]}
ROT = {0: 0, 1: 4}

U_BANDS = 5          # ceil(2432/512); 3 bands/bank at partition offsets
                     # 0/32/64 (matmul out base partition must be 0/32/64)


def _chunks():
    """Chunk list: (pieces, w_all offset) where pieces is a list of
    (slot t, ext col0, width).

    ONE combined 512-wide chunk holding the four 128-wide tails of the
    9-block slots runs FIRST (its short A+ACT fill starts the DVE stream
    ~0.9us earlier), followed by eight 1024-wide main chunks.
    """
    out = []
    tails = []
    mains = []
    for t, (ws, w) in enumerate(SLOT_WIN):
        main = min(1024, w)
        mains.append([(t, ws, main)])
        if w > main:
            tails.append((t, ws + main, w - main))
    out = [tails] + mains
    res = []
    acc = 0
    for pieces in out:
        res.append((pieces, acc))
        acc += sum(p[2] for p in pieces)
    return res


CHUNKS = _chunks()
NCOLS = sum(p[2] for c in CHUNKS for p in c[0])    # 8704


def _b_blocks(pieces, woff):
    """B matmuls for a chunk: (slot t, ext block pos, qsel, w_all col).

    One matmul per covered 128-block: stationary = the w block
    [128,128], moving = q1/q2 [128,8], out = u^T[:, 8*pos : 8*pos+8].
    q1 (weight 1) only for a slot's own diagonal block at its window
    start; everything else q2 (weight 2).
    """
    blocks = []
    off = woff
    for t, col0, cw in pieces:
        ws, _ = SLOT_WIN[t]
        for i in range(cw // 128):
            pos = col0 // 128 + i
            qsel = 1 if col0 + i * 128 == ws else 2
            blocks.append((t, pos, qsel, off + i * 128))
        off += cw
    return blocks


_compiled = None
_ops = None


def _register_ops():
    """Register the fused Ewald-weight DVE op (idempotent)."""
    global _ops
    if _ops is not None:
        return _ops
    from concourse import dve_ops
    from concourse.dve_spec import (
        Spec, Src0, Src1, C0, C1, C2, One, lower, _has_src1, minn, maxx,
    )
    from concourse.dve_uop import DveOpSpec

    def mk(name, spec):
        for o in dve_ops.OPS:
            if o.name == name:
                return o
        shas = {}
        for ver in ("v3", "v4"):
            tmp = DveOpSpec(
                name=name,
                opcode=31,
                uops=lower(spec, ver=ver),
                rd1_en=_has_src1(spec),
            )
            shas[ver] = tmp.sha(ver)
        op = dve_ops.DveOp(name, spec, subdim=False, uops_sha=shas)
        dve_ops.OPS.append(op)
        dve_ops._SUB_OPCODE_FOR_NAME[name] = (
            dve_ops._CUSTOM_DVE_ROW_BASE + len(dve_ops.OPS) - 1
        )
        dve_ops.CUSTOM_DVE_SPECS[name] = spec
        return op

    # w' = min(y, max(((C0 - s')s' + C1)s' + C2, 1))
    def _pw_ref(in0, in1, s0, s1, imm2):
        sp = in0.astype(np.float32)
        y = in1.astype(np.float32)
        p = ((np.float32(s0) - sp) * sp + np.float32(s1)) * sp + np.float32(imm2)
        return np.minimum(y, np.maximum(p, np.float32(1.0))).astype(np.float32)

    ewald_pw = mk(
        "EWALD_PW",
        Spec(
            body=minn(Src1, maxx(((C0 - Src0) * Src0 + C1) * Src0 + C2, One)),
            reference=_pw_ref,
        ),
    )
    _ops = (ewald_pw,)
    return _ops


def build_nc():
    """Build + compile the per-core Bass program (fixed shapes, SPMD)."""
    from concourse import bacc, tile
    import concourse.mybir as mybir
    from concourse.bass import ds
    from concourse.tile_rust import add_dep_helper

    (ewald_pw,) = _register_ops()
    f32 = mybir.dt.float32
    bf16 = mybir.dt.bfloat16
    AF = mybir.ActivationFunctionType

    nc = bacc.Bacc(
        "TRN2",
        target_bir_lowering=False,
        debug=False,
        num_devices=NCORES,
    )
    f16 = mybir.dt.float16
    augs = nc.dram_tensor("augs", [KA, NSLOT * 128], bf16, kind="ExternalInput").ap()
    augm = nc.dram_tensor("augm", [KA, NEXT], bf16, kind="ExternalInput").ap()
    q12 = nc.dram_tensor("q12", [128, 2 * NSLOT * NQ], bf16, kind="ExternalInput").ap()
    # u^T dump: row j (atom within block), col 8*pos+c (ext position, charge)
    uout = nc.dram_tensor("uout", [128, NPOS * NQ], f16, kind="ExternalOutput").ap()

    with tile.TileContext(nc) as tc:
        with (
            tc.tile_pool(name="const", bufs=1) as cpool,
            tc.tile_pool(name="ps", bufs=1, space="PSUM") as ps,
        ):
            # ---- input DMAs: augs and the tail-chunk's augm window
            # [1024,1536) land first (chunk 0 = combined tails); q12
            # rides the scalar queue ahead of the ACT table load.
            augs_sb = cpool.tile([KA, NSLOT * 128], bf16)
            nc.gpsimd.dma_start(out=augs_sb[:], in_=augs[:])
            augm_sb = cpool.tile([KA, NEXT], bf16)
            # one DMA for cols [0,1536): the tail chunk's window and the
            # first main chunks complete together (single DGE latency)
            nc.sync.dma_start(out=augm_sb[:, 0:1536], in_=augm[:, 0:1536])
            q12_sb = cpool.tile([128, 2 * NSLOT * NQ], bf16)
            nc.scalar.dma_start(out=q12_sb[:], in_=q12[:])
            nc.gpsimd.dma_start(out=augm_sb[:, 1536:NEXT], in_=augm[:, 1536:NEXT])

            # ---- u^T accumulator: [128, 19*8] f32 in one PSUM bank
            u_t = ps.tile([128, NPOS * NQ], f32, tag="ut", bufs=1, name="ut")
            nc.vector.memset(u_t[:], 0.0)

            # flat y/w regions (no rotation -> no reuse deps)
            y_all = cpool.tile([128, NCOLS], bf16)
            w_all = cpool.tile([128, NCOLS], bf16)

            prev_mm = [None]

            def pin(inst):
                if prev_mm[0] is not None:
                    add_dep_helper(
                        inst.ins, prev_mm[0].ins, sync=False,
                        reason="tensor queue order",
                    )
                prev_mm[0] = inst

            n_chunks = len(CHUNKS)
            s_tiles = {}

            def emit_a(k):
                pieces, _ = CHUNKS[k]
                s_ps = ps.tile([128, 1024], f32, tag="s", bufs=3, name="s_ps")
                s_tiles[k] = s_ps
                off = 0
                for t, col0, cw in pieces:
                    po = 0
                    while po < cw:
                        pw = min(512, cw - po)
                        pin(nc.tensor.matmul(
                            s_ps[:, ds(off + po, pw)],
                            augs_sb[:, ds(t * 128, 128)],
                            augm_sb[:, ds(col0 + po, pw)],
                            start=True,
                            stop=True,
                        ))
                        po += pw
                    off += cw

            def emit_act_dve(k):
                pieces, woff = CHUNKS[k]
                cw = sum(p[2] for p in pieces)
                s_ps = s_tiles[k]
                nc.scalar.activation(
                    y_all[:, ds(woff, cw)], s_ps[:, 0:cw],
                    AF.Abs_reciprocal_sqrt, scale=ACT_SCALE,
                )
                nc.vector._custom_dve(
                    ewald_pw,
                    out=w_all[:, ds(woff, cw)],
                    in0=s_ps[:, 0:cw],
                    in1=y_all[:, ds(woff, cw)],
                    s0=PC0,
                    s1=PC1,
                    imm2=PC2,
                )

            def emit_b(k):
                # stationary = w block (FWL fast load), moving = q [128,8]:
                # LDW(k+1) overlaps MM(k) in the PE reorder window, so each
                # covered block costs ~max(LDW,MM) instead of a 128-col
                # stream; u^T accumulates in one bank.
                pieces, woff = CHUNKS[k]
                for t, pos, qsel, wcol in _b_blocks(pieces, woff):
                    qoff = (0 if qsel == 1 else NSLOT * NQ) + t * NQ
                    pin(nc.tensor.matmul(
                        u_t[:, ds(pos * NQ, NQ)],
                        w_all[:, ds(wcol, 128)],
                        q12_sb[:, ds(qoff, NQ)],
                        start=False,
                        stop=False,
                        skip_group_check=True,
                    ))
                s_tiles.pop(k)

            staged = cpool.tile([128, NPOS * NQ], f16)

            LAG = 3
            for k in range(n_chunks):
                if k >= LAG:
                    emit_b(k - LAG)
                emit_a(k)
                emit_act_dve(k)
            for k in range(n_chunks - LAG, n_chunks):
                emit_b(k)

            nc.scalar.copy(staged[:], u_t[:])
            nc.sync.dma_start(out=uout[:], in_=staged[:])

    nc.compile()
    return nc


def _make_mol(rm):
    """Per-molecule hi/lo split (shared by in-map builder and host diag)."""
    import ml_dtypes

    bf = ml_dtypes.bfloat16
    rc = (rm - rm.mean(0, keepdims=True)).astype(np.float32)
    hi = rc.astype(bf)
    lo = (rc - hi.astype(np.float32)).astype(bf)
    rr = hi.astype(np.float32) + lo.astype(np.float32)
    n2 = (rr * rr).sum(1).astype(np.float32)
    n2_hi = n2.astype(bf)
    n2_lo = (n2 - n2_hi.astype(np.float32)).astype(bf)
    return hi, lo, n2_hi, n2_lo


def _aug_rows(hi, lo, n2_hi, n2_lo, rowsel):
    """The 13 (L, R) aug row pairs; R is scaled by DELTA (exact: 2^-3)."""
    import ml_dtypes

    bf = ml_dtypes.bfloat16
    n = hi.shape[0]
    dl = np.float32(DELTA)
    ones_i = np.ones(len(rowsel), bf)
    ones_j = np.full(n, dl, np.float32).astype(bf)
    rowsL, rowsR = [], []
    for ax in range(3):
        m2h = (-2.0 * hi[:, ax].astype(np.float32)).astype(bf)
        m2l = (-2.0 * lo[:, ax].astype(np.float32)).astype(bf)
        m2hd = (m2h.astype(np.float32) * dl).astype(bf)
        m2ld = (m2l.astype(np.float32) * dl).astype(bf)
        rowsL += [hi[rowsel, ax], hi[rowsel, ax], lo[rowsel, ax]]
        rowsR += [m2hd, m2ld, m2hd]
    n2_hid = (n2_hi.astype(np.float32) * dl).astype(bf)
    n2_lod = (n2_lo.astype(np.float32) * dl).astype(bf)
    rowsL += [n2_hi[rowsel], n2_lo[rowsel], ones_i, ones_i]
    rowsR += [ones_j, ones_j, n2_hid, n2_lod]
    return rowsL, rowsR


def _ext_blocks(h):
    return [(ROT[h] + p) % 16 for p in range(NPOS)]


def make_in_maps(q, r):
    """Host-side sharding: per-core augmented bf16 matrices."""
    import ml_dtypes

    bf = ml_dtypes.bfloat16
    q = np.ascontiguousarray(np.asarray(q, np.float32))
    r = np.ascontiguousarray(np.asarray(r, np.float32))
    in_maps = []
    for core in range(NCORES):
        b, h = core // 2, core % 2
        rm = r[b * NB : (b + 1) * NB]
        qm = q[b * NB : (b + 1) * NB]
        hi, lo, n2_hi, n2_lo = _make_mol(rm)

        rowsel = np.concatenate(
            [np.arange(c * 128, (c + 1) * 128) for c in CENTERS[h]]
        )
        colsel = np.concatenate(
            [np.arange(x * 128, (x + 1) * 128) for x in _ext_blocks(h)]
        )
        rowsL, rowsR = _aug_rows(hi, lo, n2_hi, n2_lo, rowsel)
        augs_np = np.ascontiguousarray(np.stack(rowsL).astype(bf))
        augm_np = np.ascontiguousarray(np.stack(rowsR)[:, colsel].astype(bf))

        qi = qm[rowsel]  # [NSLOT*128, NQ] slot-ordered
        q1 = (
            qi.reshape(NSLOT, 128, NQ).transpose(1, 0, 2).reshape(128, NSLOT * NQ)
        ).astype(bf)
        q2 = (2.0 * q1.astype(np.float32)).astype(bf)
        q12_np = np.ascontiguousarray(np.concatenate([q1, q2], axis=1))

        in_maps.append({"augs": augs_np, "augm": augm_np, "q12": q12_np})
    return in_maps


def _host_diag_w(rm):
    """Replicate the device's diagonal weight w'_ii (f32 k-ordered accum)."""
    hi, lo, n2_hi, n2_lo = _make_mol(rm)
    rowsel = np.arange(NB)
    rowsL, rowsR = _aug_rows(hi, lo, n2_hi, n2_lo, rowsel)
    eps = np.zeros(NB, np.float32)
    for L, R in zip(rowsL, rowsR):
        eps = eps + L.astype(np.float32) * R.astype(np.float32)
    eps64 = eps.astype(np.float64)
    p = ((PC0 - eps64) * eps64 + PC1) * eps64 + PC2
    with np.errstate(divide="ignore"):
        y = 1.0 / np.sqrt(np.abs(ACT_SCALE * eps64))
    return np.minimum(y, np.maximum(p, 1.0))


def _unpack_u(raw):
    """uout [128, NPOS*NQ] u^T dump -> u_ext [NQ, NEXT]."""
    ut = raw.astype(np.float64).reshape(128, NPOS, NQ)
    # u_ext[c, 128*p + j] = ut[j, p, c]
    return ut.transpose(2, 1, 0).reshape(NQ, NEXT)


def reduce_outputs(q, r, results):
    """Host-side gather: raw u banks per core -> pot[B]."""
    q = np.asarray(q, np.float32)
    r = np.asarray(r, np.float32)
    TWOPI = 2.0 * np.pi
    pots = np.zeros(B, np.float64)
    for core in range(NCORES):
        b, h = core // 2, core % 2
        u_ext = _unpack_u(results[core]["uout"])
        qm = q[b * NB : (b + 1) * NB].astype(np.float64)
        # fold extended positions back onto blocks
        u_mol = np.zeros((NQ, NB), np.float64)
        for p, x in enumerate(_ext_blocks(h)):
            u_mol[:, x * 128 : (x + 1) * 128] += u_ext[:, p * 128 : (p + 1) * 128]
        pots[b] += (u_mol * qm.T).sum()
    for b in range(B):
        rm = r[b * NB : (b + 1) * NB]
        qm = q[b * NB : (b + 1) * NB].astype(np.float64)
        wdiag = _host_diag_w(rm)
        pots[b] -= ((qm**2).sum(1) * wdiag).sum()
    pots = pots / KAPPA / (4.0 * np.pi)
    for b in range(B):
        qm = q[b * NB : (b + 1) * NB].astype(np.float64)
        pots[b] += (qm**2).sum() / ((2.0 * np.pi) ** 1.5)
    return (pots * NORM_FACTOR).astype(np.float32)


def kernel(q, r, batch):
    global _compiled
    if _compiled is None:
        _compiled = build_nc()
    from concourse import bass_utils

    in_maps = make_in_maps(q, r)
    last_err = None
    for attempt in range(3):
        try:
            res = bass_utils.run_bass_kernel_spmd(
                _compiled, in_maps, core_ids=list(range(NCORES))
            )
            return reduce_outputs(q, r, res.results)
        except Exception as e:  # transient device errors: back off and retry
            last_err = e
            import time

            time.sleep(15 * (attempt + 1))
    raise last_err


# revision 33
# speedup vs baseline: 1.0255x; 1.0255x over previous
"""Trainium2 Bass kernel for the real-space Ewald sum (nn_Ewald) — v2.

Math (per molecule, 2048 atoms, 8 charge channels):
    pot = sum_{i,j} qq_ij * erf(|rij|/sqrt(2)) / (|rij|+1e-6) / (4*pi)
        + sum_i qq_ii / (2*pi)^1.5   (self term), scaled by NORM_FACTOR.

Per pair tile the device computes (as in v1):
    s' = delta*s  (K=13 bf16 hi/lo augmented matmul, delta=2^-3 exact)
    y  = kappa/sqrt(s)        (ACT Abs_reciprocal_sqrt -> bf16)
    w' = min(y, max(P(s'),1)) (one custom DVE op, monic cubic P -> bf16)
    u[c,j] += q[i,c] w'_ij    (PE contraction, f32 PSUM)
Host: pot = (sum(u*q^T) - sum_i qq_ii w'_ii)/kappa/(4*pi) + self term.

v2 geometry — circulant tournament cover at 128-atom granularity:
  16 blocks per molecule; slot centered at block c covers the cyclic
  block window [c, c+8] (9 blocks) for c in 0..7, [c, c+7] (8 blocks)
  for c in 8..15.  Every unordered block pair is covered exactly once
  (difference 1..7 -> lower center; difference 8 -> center < 8; diag ->
  own center with weight 1 via q1, off-center blocks weight 2 via q2).
  Total 68 blocks/core = 8704 pair-columns (vs 10240 in v1, -15%).

SPMD: every core runs slots of widths {9,9,9,9,8,8,8,8} at FIXED window
positions [128j, 128j+1152) (j=0..3) and [128(8+j), 128(8+j)+1024) over
a 19-block extended column space.  Per-core data: augm_ext columns are
the molecule's blocks in cyclic order rotated by 0 (even core: centers
0-3,8-11) or 4 (odd core: centers 4-7,12-15); augs/q12 hold the 8
center blocks.  8 cores = 4 molecules x 2 center sets.

u accumulates in 2 PSUM banks (bands of 512 columns packed at partition
offsets 0/32/64/96 via matmul tile_position); banks are memset to zero
so all matmuls accumulate (start=False).  Host folds the 19-position
extended u back onto the 16 blocks.

PE runs at 1.2 GHz on this platform (HAM clock gate never releases), so
the kernel is PE-stream-bound: A 8704 + B 8704 columns ~= 14.5 us/core.
"""

import numpy as np

B = 4
NB = 2048
NQ = 8
NCORES = 8
NSLOT = 8
NPOS = 19            # extended column space, in 128-blocks
NEXT = NPOS * 128    # 2432 extended columns
NORM_FACTOR = 90.0474
KA = 13              # augmented contraction depth (bf16 hi/lo split)

# fitted scheme constants (see v1): s_c=9, delta=2^-3
DELTA = 0.125
KAPPA = 3.0708577931200534
PC0 = 2.7432632222505378
PC1 = -3.112066562880879
PC2 = 2.4529603188601343
ACT_SCALE = 1.0 / (DELTA * KAPPA * KAPPA)

# slot windows in the extended space: (start, width) in columns
SLOT_WIN = [(128 * j, 1152) for j in range(4)] + [
    (128 * (8 + j), 1024) for j in range(4)
]
# centers per core half h (h=0: rotation 0, h=1: rotation 4)
CENTERS = {0: [0, 1, 2, 3, 8, 9, 10, 11], 1: [4, 5, 6, 7, 12, 13, 14, 15]}
ROT = {0: 0, 1: 4}

U_BANDS = 5          # ceil(2432/512); 3 bands/bank at partition offsets
                     # 0/32/64 (matmul out base partition must be 0/32/64)


def _chunks():
    """Chunk list: (pieces, w_all offset) where pieces is a list of
    (slot t, ext col0, width).

    ONE combined 512-wide chunk holding the four 128-wide tails of the
    9-block slots runs FIRST (its short A+ACT fill starts the DVE stream
    ~0.9us earlier), followed by eight 1024-wide main chunks.
    """
    out = []
    tails = []
    mains = []
    for t, (ws, w) in enumerate(SLOT_WIN):
        main = min(1024, w)
        mains.append([(t, ws, main)])
        if w > main:
            tails.append((t, ws + main, w - main))
    out = [tails] + mains
    res = []
    acc = 0
    for pieces in out:
        res.append((pieces, acc))
        acc += sum(p[2] for p in pieces)
    return res


CHUNKS = _chunks()
NCOLS = sum(p[2] for c in CHUNKS for p in c[0])    # 8704


def _b_blocks(pieces, woff):
    """B matmuls for a chunk: (slot t, ext block pos, qsel, w_all col).

    One matmul per covered 128-block: stationary = the w block
    [128,128], moving = q1/q2 [128,8], out = u^T[:, 8*pos : 8*pos+8].
    q1 (weight 1) only for a slot's own diagonal block at its window
    start; everything else q2 (weight 2).
    """
    blocks = []
    off = woff
    for t, col0, cw in pieces:
        ws, _ = SLOT_WIN[t]
        for i in range(cw // 128):
            pos = col0 // 128 + i
            qsel = 1 if col0 + i * 128 == ws else 2
            blocks.append((t, pos, qsel, off + i * 128))
        off += cw
    return blocks


_compiled = None
_ops = None


def _register_ops():
    """Register the fused Ewald-weight DVE op (idempotent)."""
    global _ops
    if _ops is not None:
        return _ops
    from concourse import dve_ops
    from concourse.dve_spec import (
        Spec, Src0, Src1, C0, C1, C2, One, lower, _has_src1, minn, maxx,
    )
    from concourse.dve_uop import DveOpSpec

    def mk(name, spec):
        for o in dve_ops.OPS:
            if o.name == name:
                return o
        shas = {}
        for ver in ("v3", "v4"):
            tmp = DveOpSpec(
                name=name,
                opcode=31,
                uops=lower(spec, ver=ver),
                rd1_en=_has_src1(spec),
            )
            shas[ver] = tmp.sha(ver)
        op = dve_ops.DveOp(name, spec, subdim=False, uops_sha=shas)
        dve_ops.OPS.append(op)
        dve_ops._SUB_OPCODE_FOR_NAME[name] = (
            dve_ops._CUSTOM_DVE_ROW_BASE + len(dve_ops.OPS) - 1
        )
        dve_ops.CUSTOM_DVE_SPECS[name] = spec
        return op

    # w' = min(y, max(((C0 - s')s' + C1)s' + C2, 1))
    def _pw_ref(in0, in1, s0, s1, imm2):
        sp = in0.astype(np.float32)
        y = in1.astype(np.float32)
        p = ((np.float32(s0) - sp) * sp + np.float32(s1)) * sp + np.float32(imm2)
        return np.minimum(y, np.maximum(p, np.float32(1.0))).astype(np.float32)

    ewald_pw = mk(
        "EWALD_PW",
        Spec(
            body=minn(Src1, maxx(((C0 - Src0) * Src0 + C1) * Src0 + C2, One)),
            reference=_pw_ref,
        ),
    )
    _ops = (ewald_pw,)
    return _ops


def build_nc():
    """Build + compile the per-core Bass program (fixed shapes, SPMD)."""
    from concourse import bacc, tile
    import concourse.mybir as mybir
    from concourse.bass import ds
    from concourse.tile_rust import add_dep_helper

    (ewald_pw,) = _register_ops()
    f32 = mybir.dt.float32
    bf16 = mybir.dt.bfloat16
    AF = mybir.ActivationFunctionType

    nc = bacc.Bacc(
        "TRN2",
        target_bir_lowering=False,
        debug=False,
        num_devices=NCORES,
    )
    f16 = mybir.dt.float16
    augs = nc.dram_tensor("augs", [KA, NSLOT * 128], bf16, kind="ExternalInput").ap()
    augm = nc.dram_tensor("augm", [KA, NEXT], bf16, kind="ExternalInput").ap()
    q12 = nc.dram_tensor("q12", [128, 2 * NSLOT * NQ], bf16, kind="ExternalInput").ap()
    # u^T dump: row j (atom within block), col 8*pos+c (ext position, charge)
    uout = nc.dram_tensor("uout", [128, NPOS * NQ], f16, kind="ExternalOutput").ap()

    with tile.TileContext(nc) as tc:
        with (
            tc.tile_pool(name="const", bufs=1) as cpool,
            tc.tile_pool(name="ps", bufs=1, space="PSUM") as ps,
        ):
            # ---- input DMAs: augs and the tail-chunk's augm window
            # [1024,1536) land first (chunk 0 = combined tails); q12
            # rides the scalar queue ahead of the ACT table load.
            augs_sb = cpool.tile([KA, NSLOT * 128], bf16)
            nc.gpsimd.dma_start(out=augs_sb[:], in_=augs[:])
            augm_sb = cpool.tile([KA, NEXT], bf16)
            nc.sync.dma_start(out=augm_sb[:, 1024:1536], in_=augm[:, 1024:1536])
            q12_sb = cpool.tile([128, 2 * NSLOT * NQ], bf16)
            nc.scalar.dma_start(out=q12_sb[:], in_=q12[:])
            nc.sync.dma_start(out=augm_sb[:, 0:1024], in_=augm[:, 0:1024])
            nc.gpsimd.dma_start(out=augm_sb[:, 1536:NEXT], in_=augm[:, 1536:NEXT])

            # ---- u^T accumulator: [128, 19*8] f32 in one PSUM bank
            u_t = ps.tile([128, NPOS * NQ], f32, tag="ut", bufs=1, name="ut")
            nc.vector.memset(u_t[:], 0.0)

            # flat y/w regions (no rotation -> no reuse deps)
            y_all = cpool.tile([128, NCOLS], bf16)
            w_all = cpool.tile([128, NCOLS], bf16)

            prev_mm = [None]

            def pin(inst):
                if prev_mm[0] is not None:
                    add_dep_helper(
                        inst.ins, prev_mm[0].ins, sync=False,
                        reason="tensor queue order",
                    )
                prev_mm[0] = inst

            n_chunks = len(CHUNKS)
            s_tiles = {}

            def emit_a(k):
                pieces, _ = CHUNKS[k]
                s_ps = ps.tile([128, 1024], f32, tag="s", bufs=3, name="s_ps")
                s_tiles[k] = s_ps
                off = 0
                for t, col0, cw in pieces:
                    po = 0
                    while po < cw:
                        pw = min(512, cw - po)
                        pin(nc.tensor.matmul(
                            s_ps[:, ds(off + po, pw)],
                            augs_sb[:, ds(t * 128, 128)],
                            augm_sb[:, ds(col0 + po, pw)],
                            start=True,
                            stop=True,
                        ))
                        po += pw
                    off += cw

            def emit_act_dve(k):
                pieces, woff = CHUNKS[k]
                cw = sum(p[2] for p in pieces)
                s_ps = s_tiles[k]
                nc.scalar.activation(
                    y_all[:, ds(woff, cw)], s_ps[:, 0:cw],
                    AF.Abs_reciprocal_sqrt, scale=ACT_SCALE,
                )
                nc.vector._custom_dve(
                    ewald_pw,
                    out=w_all[:, ds(woff, cw)],
                    in0=s_ps[:, 0:cw],
                    in1=y_all[:, ds(woff, cw)],
                    s0=PC0,
                    s1=PC1,
                    imm2=PC2,
                )

            def emit_b(k):
                # stationary = w block (FWL fast load), moving = q [128,8]:
                # LDW(k+1) overlaps MM(k) in the PE reorder window, so each
                # covered block costs ~max(LDW,MM) instead of a 128-col
                # stream; u^T accumulates in one bank.
                pieces, woff = CHUNKS[k]
                for t, pos, qsel, wcol in _b_blocks(pieces, woff):
                    qoff = (0 if qsel == 1 else NSLOT * NQ) + t * NQ
                    pin(nc.tensor.matmul(
                        u_t[:, ds(pos * NQ, NQ)],
                        w_all[:, ds(wcol, 128)],
                        q12_sb[:, ds(qoff, NQ)],
                        start=False,
                        stop=False,
                        skip_group_check=True,
                    ))
                s_tiles.pop(k)

            staged = cpool.tile([128, NPOS * NQ], f16)

            LAG = 3
            for k in range(n_chunks):
                if k >= LAG:
                    emit_b(k - LAG)
                emit_a(k)
                emit_act_dve(k)
            for k in range(n_chunks - LAG, n_chunks):
                emit_b(k)

            nc.scalar.copy(staged[:], u_t[:])
            nc.sync.dma_start(out=uout[:], in_=staged[:])

    nc.compile()
    return nc


def _make_mol(rm):
    """Per-molecule hi/lo split (shared by in-map builder and host diag)."""
    import ml_dtypes

    bf = ml_dtypes.bfloat16
    rc = (rm - rm.mean(0, keepdims=True)).astype(np.float32)
    hi = rc.astype(bf)
    lo = (rc - hi.astype(np.float32)).astype(bf)
    rr = hi.astype(np.float32) + lo.astype(np.float32)
    n2 = (rr * rr).sum(1).astype(np.float32)
    n2_hi = n2.astype(bf)
    n2_lo = (n2 - n2_hi.astype(np.float32)).astype(bf)
    return hi, lo, n2_hi, n2_lo


def _aug_rows(hi, lo, n2_hi, n2_lo, rowsel):
    """The 13 (L, R) aug row pairs; R is scaled by DELTA (exact: 2^-3)."""
    import ml_dtypes

    bf = ml_dtypes.bfloat16
    n = hi.shape[0]
    dl = np.float32(DELTA)
    ones_i = np.ones(len(rowsel), bf)
    ones_j = np.full(n, dl, np.float32).astype(bf)
    rowsL, rowsR = [], []
    for ax in range(3):
        m2h = (-2.0 * hi[:, ax].astype(np.float32)).astype(bf)
        m2l = (-2.0 * lo[:, ax].astype(np.float32)).astype(bf)
        m2hd = (m2h.astype(np.float32) * dl).astype(bf)
        m2ld = (m2l.astype(np.float32) * dl).astype(bf)
        rowsL += [hi[rowsel, ax], hi[rowsel, ax], lo[rowsel, ax]]
        rowsR += [m2hd, m2ld, m2hd]
    n2_hid = (n2_hi.astype(np.float32) * dl).astype(bf)
    n2_lod = (n2_lo.astype(np.float32) * dl).astype(bf)
    rowsL += [n2_hi[rowsel], n2_lo[rowsel], ones_i, ones_i]
    rowsR += [ones_j, ones_j, n2_hid, n2_lod]
    return rowsL, rowsR


def _ext_blocks(h):
    return [(ROT[h] + p) % 16 for p in range(NPOS)]


def make_in_maps(q, r):
    """Host-side sharding: per-core augmented bf16 matrices."""
    import ml_dtypes

    bf = ml_dtypes.bfloat16
    q = np.ascontiguousarray(np.asarray(q, np.float32))
    r = np.ascontiguousarray(np.asarray(r, np.float32))
    in_maps = []
    for core in range(NCORES):
        b, h = core // 2, core % 2
        rm = r[b * NB : (b + 1) * NB]
        qm = q[b * NB : (b + 1) * NB]
        hi, lo, n2_hi, n2_lo = _make_mol(rm)

        rowsel = np.concatenate(
            [np.arange(c * 128, (c + 1) * 128) for c in CENTERS[h]]
        )
        colsel = np.concatenate(
            [np.arange(x * 128, (x + 1) * 128) for x in _ext_blocks(h)]
        )
        rowsL, rowsR = _aug_rows(hi, lo, n2_hi, n2_lo, rowsel)
        augs_np = np.ascontiguousarray(np.stack(rowsL).astype(bf))
        augm_np = np.ascontiguousarray(np.stack(rowsR)[:, colsel].astype(bf))

        qi = qm[rowsel]  # [NSLOT*128, NQ] slot-ordered
        q1 = (
            qi.reshape(NSLOT, 128, NQ).transpose(1, 0, 2).reshape(128, NSLOT * NQ)
        ).astype(bf)
        q2 = (2.0 * q1.astype(np.float32)).astype(bf)
        q12_np = np.ascontiguousarray(np.concatenate([q1, q2], axis=1))

        in_maps.append({"augs": augs_np, "augm": augm_np, "q12": q12_np})
    return in_maps


def _host_diag_w(rm):
    """Replicate the device's diagonal weight w'_ii (f32 k-ordered accum)."""
    hi, lo, n2_hi, n2_lo = _make_mol(rm)
    rowsel = np.arange(NB)
    rowsL, rowsR = _aug_rows(hi, lo, n2_hi, n2_lo, rowsel)
    eps = np.zeros(NB, np.float32)
    for L, R in zip(rowsL, rowsR):
        eps = eps + L.astype(np.float32) * R.astype(np.float32)
    eps64 = eps.astype(np.float64)
    p = ((PC0 - eps64) * eps64 + PC1) * eps64 + PC2
    with np.errstate(divide="ignore"):
        y = 1.0 / np.sqrt(np.abs(ACT_SCALE * eps64))
    return np.minimum(y, np.maximum(p, 1.0))


def _unpack_u(raw):
    """uout [128, NPOS*NQ] u^T dump -> u_ext [NQ, NEXT]."""
    ut = raw.astype(np.float64).reshape(128, NPOS, NQ)
    # u_ext[c, 128*p + j] = ut[j, p, c]
    return ut.transpose(2, 1, 0).reshape(NQ, NEXT)


def reduce_outputs(q, r, results):
    """Host-side gather: raw u banks per core -> pot[B]."""
    q = np.asarray(q, np.float32)
    r = np.asarray(r, np.float32)
    TWOPI = 2.0 * np.pi
    pots = np.zeros(B, np.float64)
    for core in range(NCORES):
        b, h = core // 2, core % 2
        u_ext = _unpack_u(results[core]["uout"])
        qm = q[b * NB : (b + 1) * NB].astype(np.float64)
        # fold extended positions back onto blocks
        u_mol = np.zeros((NQ, NB), np.float64)
        for p, x in enumerate(_ext_blocks(h)):
            u_mol[:, x * 128 : (x + 1) * 128] += u_ext[:, p * 128 : (p + 1) * 128]
        pots[b] += (u_mol * qm.T).sum()
    for b in range(B):
        rm = r[b * NB : (b + 1) * NB]
        qm = q[b * NB : (b + 1) * NB].astype(np.float64)
        wdiag = _host_diag_w(rm)
        pots[b] -= ((qm**2).sum(1) * wdiag).sum()
    pots = pots / KAPPA / (4.0 * np.pi)
    for b in range(B):
        qm = q[b * NB : (b + 1) * NB].astype(np.float64)
        pots[b] += (qm**2).sum() / ((2.0 * np.pi) ** 1.5)
    return (pots * NORM_FACTOR).astype(np.float32)


def kernel(q, r, batch):
    global _compiled
    if _compiled is None:
        _compiled = build_nc()
    from concourse import bass_utils

    in_maps = make_in_maps(q, r)
    last_err = None
    for attempt in range(3):
        try:
            res = bass_utils.run_bass_kernel_spmd(
                _compiled, in_maps, core_ids=list(range(NCORES))
            )
            return reduce_outputs(q, r, res.results)
        except Exception as e:  # transient device errors: back off and retry
            last_err = e
            import time

            time.sleep(15 * (attempt + 1))
    raise last_err
